# revision 8
# baseline (speedup 1.0000x reference)
"""Trainium2 Bass kernel for nn_Attention_63273458205325.

Data-parallel over batch: 64 images -> 8 NeuronCores x 8 images.
Device kernel computes, per image, the four memory-bound global
reductions over x[b] (256x4096 fp32):
  - beta row-sums  (per-channel sum over spatial)       [256]
  - mask logits m = w_mask . x  -> exp -> Z and the
    softmax-weighted context sums  sum_s x[c,s]*e[s]    [256]
  - mean over spatial of (max over channels)            scalar
The tiny [B,8] epilogue head runs on host.

v3 engine balance per image (fp32 HBM loads = the 94us/core roofline):
  ACT : cast x0->bf16 with rowsum accum + exp (4x[1,1024])
  Pool: cast x1->bf16 with rowsum accum (gpsimd tensor_scalar)
  DVE : ctx stt from bf16 PSUM e-broadcast (2x hoped) + max fold +
        ct max-reduces + small reduces
  PE  : 16 logits MMs + 32 max transposes + 4 e-broadcast MMs
        (bf16 PSUM out via transpose-mode matmul)
Emission is software-pipelined: image b's bcast/stt phase is emitted
after image b+1's fold/transpose phase so no engine queue stalls on a
cross-engine dependency.
"""

import sys

import numpy as np

sys.path.insert(0, "/opt/trn_rl_repo")

B, C, H, W = 64, 256, 64, 64
S = H * W  # 4096
NCORES = 8
BPC = B // NCORES  # images per core
RATIO, K = 16, 8
PLANES = C // 2
HIDDEN = C // RATIO
TEMP = 30.0
EPS = 1e-5

# e-broadcast PSUM dtype: bf16 (via transpose-mode matmul) enables packed
# 2x DVE reads in the ctx stt; set False to fall back to fp32 PSUM.
EB_BF16 = False
# engine for the x1 cast+rowsum: "gpsimd" (idle engine) or "vector"
X1_CAST_ENGINE = "gpsimd"

_CACHE = {}


def _build_nc():
    import concourse.bacc as bacc
    import concourse.mybir as mybir
    from concourse.tile import TileContext

    f32 = mybir.dt.float32
    bf16 = mybir.dt.bfloat16
    AF = mybir.ActivationFunctionType
    ALU = mybir.AluOpType
    AX = mybir.AxisListType

    nc = bacc.Bacc(None, target_bir_lowering=False)

    x_ext = nc.declare_dram_parameter("x", [BPC, C, S], f32, isOutput=False)
    wm_ext = nc.declare_dram_parameter("wm", [C], bf16, isOutput=False)
    ones_ext = nc.declare_dram_parameter("ones1", [1, 128], bf16, isOutput=False)
    id_ext = nc.declare_dram_parameter("ident", [128, 128], bf16, isOutput=False)
    out_ext = nc.declare_dram_parameter("out", [BPC, 128, 8], f32, isOutput=True)

    eb_dt = bf16 if EB_BF16 else f32
    eb_cols = 2048
    mm_cols = 1024 if EB_BF16 else 512  # bcast matmul width (1 PSUM bank)

    with TileContext(nc) as tc:
        with (
            tc.tile_pool(name="const", bufs=1) as cpool,
            tc.tile_pool(name="xf32", bufs=2) as fpool,
            tc.tile_pool(name="xin", bufs=2) as xpool,
            tc.tile_pool(name="ework", bufs=2) as epool,
            tc.tile_pool(name="junk", bufs=2) as jpool,
            tc.tile_pool(name="pmax", bufs=2) as mpool,
            tc.tile_pool(name="small", bufs=3) as spool,
            tc.tile_pool(name="psum", bufs=2, space="PSUM") as ppool,
            tc.tile_pool(name="psum1", bufs=1, space="PSUM") as p1pool,
        ):
            # constants
            wm = cpool.tile([128, 2], bf16)  # wm[p, g] = w_mask[g*128 + p]
            nc.sync.dma_start(out=wm[:], in_=wm_ext.rearrange("(g p) -> p g", p=128))
            ones1 = cpool.tile([1, 128], bf16)
            nc.sync.dma_start(out=ones1[:], in_=ones_ext[:])
            ident = cpool.tile([128, 128], bf16)
            nc.sync.dma_start(out=ident[:], in_=id_ext[:])

            prev = None
            for b in range(BPC + 1):
                cur = None
                if b < BPC:
                    # ---------- phase A for image b ----------
                    x0 = fpool.tile([128, S], f32, tag="x0")
                    nc.sync.dma_start(out=x0[:], in_=x_ext[b, 0:128, :])
                    x1 = fpool.tile([128, S], f32, tag="x1")
                    nc.sync.dma_start(out=x1[:], in_=x_ext[b, 128:256, :])

                    stage = spool.tile([128, 8], f32, tag="stage")
                    nc.gpsimd.memset(stage[:], 0.0)

                    # casts with rowsum accumulate riding free
                    xb0 = xpool.tile([128, S], bf16, tag="xb0")
                    nc.scalar.activation(xb0[:], x0[:], AF.Copy,
                                         accum_out=stage[:, 0:1])
                    xb1 = xpool.tile([128, S], bf16, tag="xb1")
                    if X1_CAST_ENGINE == "gpsimd":
                        nc.gpsimd.tensor_scalar(out=xb1[:], in0=x1[:],
                                                scalar1=1.0, scalar2=None,
                                                op0=ALU.mult)
                        jk = jpool.tile([128, S], bf16, tag="jk")
                        nc.scalar.activation(jk[:], xb1[:], AF.Copy,
                                             accum_out=stage[:, 1:2])
                    else:
                        nc.vector.tensor_scalar(out=xb1[:], in0=x1[:],
                                                scalar1=1.0, scalar2=0.0,
                                                op0=ALU.mult, op1=ALU.add,
                                                accum_out=stage[:, 1:2])

                    # mask logits -> exp -> e_row, Z partials
                    e_row = epool.tile([1, S], bf16, tag="e")
                    zacc = spool.tile([1, 1], f32, tag="z")
                    zacc8 = spool.tile([1, 8], f32, tag="z8")
                    for q in range(8):
                        m_ps = ppool.tile([1, 512], f32, tag="m")
                        gl = slice(512 * q, 512 * (q + 1))
                        nc.tensor.matmul(m_ps[:], lhsT=wm[:, 0:1],
                                         rhs=xb0[:, gl], start=True, stop=False)
                        nc.tensor.matmul(m_ps[:], lhsT=wm[:, 1:2],
                                         rhs=xb1[:, gl], start=False, stop=True)
                        nc.scalar.activation(e_row[:, gl], m_ps[:], AF.Exp,
                                             accum_out=zacc8[:, q:q + 1])
                    nc.vector.tensor_reduce(zacc[:, 0:1], zacc8[:], axis=AX.X,
                                            op=ALU.add)

                    # channel max: fold 256->128, transpose, free-axis reduce
                    pm = mpool.tile([128, S], bf16, tag="pm")
                    nc.vector.tensor_max(pm[:], xb0[:], xb1[:])
                    rm = spool.tile([128, 32], bf16, tag="rm")
                    for g in range(4):
                        ct_ps = ppool.tile([128, 1024], bf16, tag="ct")
                        for j in range(8):
                            cj = 8 * g + j
                            nc.tensor.transpose(ct_ps[:, 128 * j:128 * (j + 1)],
                                                pm[:, 128 * cj:128 * (cj + 1)],
                                                ident[:])
                        nc.vector.tensor_reduce(
                            rm[:, 8 * g:8 * (g + 1)],
                            ct_ps[:].rearrange("p (j c) -> p j c", c=128),
                            axis=AX.X, op=ALU.max)

                    cur = dict(stage=stage, e_row=e_row, zacc=zacc, rm=rm,
                               xb0=xb0, xb1=xb1, bidx=b)

                if prev is not None:
                    # ---------- phase B for image b-1 ----------
                    st, er = prev["stage"], prev["e_row"]
                    nh = S // eb_cols
                    cacc = spool.tile([128, 2 * nh], f32, tag="cacc")
                    scr = jpool.tile([128, eb_cols], bf16, tag="scr")
                    for h in range(S // eb_cols):
                        eb_ps = p1pool.tile([128, eb_cols], eb_dt, tag="eb")
                        for u in range(eb_cols // mm_cols):
                            sl = slice(eb_cols * h + mm_cols * u,
                                       eb_cols * h + mm_cols * (u + 1))
                            pl = slice(mm_cols * u, mm_cols * (u + 1))
                            nc.tensor.matmul(eb_ps[:, pl], lhsT=ones1[:],
                                             rhs=er[:, sl], start=True,
                                             stop=True)
                        for ci, xbh in ((0, prev["xb0"]), (1, prev["xb1"])):
                            nc.vector.scalar_tensor_tensor(
                                out=scr[:],
                                in0=xbh[:, eb_cols * h:eb_cols * (h + 1)],
                                scalar=1.0, in1=eb_ps[:],
                                op0=ALU.mult, op1=ALU.mult,
                                accum_out=cacc[:, ci * nh + h:ci * nh + h + 1])
                    nc.vector.tensor_reduce(
                        st[:, 2:4],
                        cacc[:].rearrange("p (c j) -> p c j", j=nh),
                        axis=AX.X, op=ALU.add)
                    nc.vector.tensor_reduce(st[:, 4:5], prev["rm"][:],
                                            axis=AX.X, op=ALU.add)
                    nc.vector.tensor_scalar_add(st[0:1, 5:6],
                                                prev["zacc"][:], 0.0)
                    nc.sync.dma_start(out=out_ext[prev["bidx"]], in_=st[:])

                prev = cur
    return nc


def _get_nc():
    if "nc" not in _CACHE:
        nc = _build_nc()
        nc.finalize()
        _CACHE["nc"] = nc
    return _CACHE["nc"]


def _run_device(x_np, trace=False, tmpdir=None):
    """x_np: [64, 256, 64, 64] fp32 -> list of 8 per-core result dicts."""
    import ml_dtypes
    from concourse.bass_utils import run_bass_kernel_spmd

    nc = _get_nc()
    xs = x_np.reshape(NCORES, BPC, C, S)
    wm = _CACHE["w_mask"].reshape(C).astype(ml_dtypes.bfloat16)
    ones1 = np.ones([1, 128], dtype=ml_dtypes.bfloat16)
    ident = np.eye(128, dtype=ml_dtypes.bfloat16)
    in_maps = [
        {"x": np.ascontiguousarray(xs[i]), "wm": wm, "ones1": ones1, "ident": ident}
        for i in range(NCORES)
    ]
    res = run_bass_kernel_spmd(nc, in_maps, core_ids=list(range(NCORES)),
                               trace=trace, tmpdir=tmpdir)
    return res


def kernel(x, w_mask, b_mask, w_cm1, b_cm1, ln_w, ln_b, w_cm2, b_cm2,
           w_net1, w_net2, w_fc, bn_w, bn_b, bn_mean, bn_var, w_kfc):
    x = np.asarray(x, dtype=np.float32)
    _CACHE["w_mask"] = np.asarray(w_mask, dtype=np.float32)
    res = _run_device(x)

    # ---- gather device results
    beta_sums = np.zeros([B, C], np.float32)
    ctx_sums = np.zeros([B, C], np.float32)
    zs = np.zeros([B], np.float32)
    cmax_sums = np.zeros([B], np.float32)
    for i in range(NCORES):
        o = np.asarray(res.results[i]["out"], np.float32)  # [BPC, 128, 8]
        for bb in range(BPC):
            g = i * BPC + bb
            beta_sums[g, 0:128] = o[bb, :, 0]
            beta_sums[g, 128:256] = o[bb, :, 1]
            ctx_sums[g, 0:128] = o[bb, :, 2]
            ctx_sums[g, 128:256] = o[bb, :, 3]
            cmax_sums[g] = o[bb, :, 4].sum()
            zs[g] = o[bb, 0, 5]

    # ---- tiny epilogue head on host (mirrors reference.py)
    w_cm1 = np.asarray(w_cm1, np.float32); b_cm1 = np.asarray(b_cm1, np.float32)
    ln_w = np.asarray(ln_w, np.float32); ln_b = np.asarray(ln_b, np.float32)
    w_cm2 = np.asarray(w_cm2, np.float32); b_cm2 = np.asarray(b_cm2, np.float32)
    w_net1 = np.asarray(w_net1, np.float32); w_net2 = np.asarray(w_net2, np.float32)
    w_fc = np.asarray(w_fc, np.float32); bn_w = np.asarray(bn_w, np.float32)
    bn_b = np.asarray(bn_b, np.float32); bn_mean = np.asarray(bn_mean, np.float32)
    bn_var = np.asarray(bn_var, np.float32); w_kfc = np.asarray(w_kfc, np.float32)

    from scipy.special import erf  # exact gelu, matches jax approximate=False

    beta_c = beta_sums / S
    context = ctx_sums / zs[:, None]
    a = beta_sums.sum(axis=1) / (C * S)
    mm = cmax_sums / S
    beta_s = np.zeros([B, C], np.float32)
    beta_s[:, 0::2] = a[:, None]
    beta_s[:, 1::2] = mm[:, None]

    t = context @ w_cm1.T + b_cm1
    mu = t.mean(axis=-1, keepdims=True)
    var = ((t - mu) ** 2).mean(axis=-1, keepdims=True)
    t = (t - mu) / np.sqrt(var + EPS) * ln_w + ln_b
    t = t * 0.5 * (1.0 + erf(t / np.sqrt(2.0)))
    beta_g = t @ w_cm2.T + b_cm2

    out = beta_c + beta_g + beta_s
    out = np.maximum(out @ w_net1.T, 0.0) @ w_net2.T  # [B, K]

    ka = out @ w_fc.T
    ka = (ka - bn_mean) / np.sqrt(bn_var + EPS) * bn_w + bn_b
    kat = 1.0 / (1.0 + np.exp(-(np.maximum(ka, 0.0) @ w_kfc.T)))
    out = out * kat
    out = out / TEMP
    out = out - out.max(axis=-1, keepdims=True)
    e = np.exp(out)
    return (e / e.sum(axis=-1, keepdims=True)).astype(np.float32)
